# revision 7
# baseline (speedup 1.0000x reference)
"""Trainium2 Bass kernel for nn_BoxHead (nms_detection).

reference:
    h  = relu(X @ W1 + b1)         X: [2000, 50176], W1: [50176, 1024]
    h  = relu(h @ W2 + b2)         W2: [1024, 1024]
    cl = h @ Wc + bc               Wc: [1024, 4]
    bx = h @ Wr + br               Wr: [1024, 12]
    return (cl, bx)

Sharding: contraction-dim (K) split of layer 1 across 8 NeuronCores.
Each core holds X.T rows [6272c:6272(c+1)] (host-transposed, row-padded
to 2048) and the matching W1 row-slice, computes the partial product
partial_c = X_kslice @ W1_kslice -> [2048, 1024] in two row-phases of
1024. After each phase a ReduceScatter(add) leaves each core with 128
fully-reduced rows per phase (256 rows total), on which it runs
bias+relu, layer 2, and both heads (concatenated into one [1024, 16]
weight). The tiny head outputs are returned per-core and stitched on
the host.

Matmuls run as float32r (fp32 storage, single-pass "HIGH" PE mode):
full fp32 DMA bytes, ~4x the fp32 matmul throughput, ~1e-4 rel err.
"""

import numpy as np

NCORES = 8
D_IN = 50176
KC = D_IN // NCORES      # 6272 K-slice per core
KT = KC // 128           # 49 k-tiles of 128
ROWS = 2000
ROWS_PAD = 2048
HID = 1024
N_PHASES = 2
M_TILES = ROWS_PAD // 128            # 16
M_PER_PHASE = M_TILES // N_PHASES    # 8
PHASE_ROWS = 128 * M_PER_PHASE       # 1024
RS_ROWS = PHASE_ROWS // NCORES       # 128 rows per (core, phase)
LOCAL_ROWS = RS_ROWS * N_PHASES      # 256 rows handled per core
K_CACHE = 8                          # k-tiles of W1 kept resident in SBUF
CHUNK = 7                            # k-tiles per PSUM accumulation chunk
N_CHUNKS = KT // CHUNK               # 7
C = 3
N_HEAD = 16                          # 4 class logits + 12 box deltas

_CACHE = {}


def _build():
    import concourse.mybir as mybir
    import concourse.tile as tile
    from concourse import bacc
    from concourse.masks import make_identity

    F32 = mybir.dt.float32
    F32R = mybir.dt.float32r
    RELU = mybir.ActivationFunctionType.Relu
    IDENT = mybir.ActivationFunctionType.Identity

    nc = bacc.Bacc(None, target_bir_lowering=False, debug=False, num_devices=NCORES)

    xT = nc.dram_tensor("xT", [KC, ROWS_PAD], F32R, kind="ExternalInput")
    w1 = nc.dram_tensor("w1", [KC, HID], F32R, kind="ExternalInput")
    w2 = nc.dram_tensor("w2", [HID, HID], F32R, kind="ExternalInput")
    wh = nc.dram_tensor("wh", [HID, N_HEAD], F32R, kind="ExternalInput")
    b1 = nc.dram_tensor("b1", [HID], F32, kind="ExternalInput")
    b2 = nc.dram_tensor("b2", [HID], F32, kind="ExternalInput")
    bh = nc.dram_tensor("bh", [N_HEAD], F32, kind="ExternalInput")
    out = nc.dram_tensor("out", [N_HEAD, LOCAL_ROWS], F32, kind="ExternalOutput")

    xT3 = xT.rearrange("(ko p) m -> p ko m", p=128)    # [128, 49, 2048]
    w13 = w1.rearrange("(ko p) n -> p ko n", p=128)    # [128, 49, 1024]
    w23 = w2.rearrange("(o p) n -> p o n", p=128)      # [128, 8, 1024]
    wh3 = wh.rearrange("(o p) n -> p o n", p=128)      # [128, 8, 16]

    with tile.TileContext(nc) as tc:
        with (
            tc.tile_pool(name="const", bufs=1) as const,
            tc.tile_pool(name="w1s", bufs=10) as w1s,
            tc.tile_pool(name="xts", bufs=5) as xts,
            tc.tile_pool(name="w2s", bufs=2) as w2s,
            tc.tile_pool(name="psmm", bufs=3, space="PSUM") as psmm,
            tc.tile_pool(name="psmisc", bufs=2, space="PSUM") as psmisc,
            tc.tile_pool(name="dram", bufs=1, space="DRAM") as dram,
        ):
            # ---- persistent SBUF ----
            w1c = const.tile([128, K_CACHE, HID], F32R)            # 28KB/part
            accum = [
                const.tile([128, M_PER_PHASE, HID], F32, name=f"accum{p}")
                for p in range(N_PHASES)
            ]
            hT = const.tile([128, 8, LOCAL_ROWS], F32R)            # 8KB/part
            h2T = const.tile([128, 8, LOCAL_ROWS], F32R)           # 8KB/part
            rs_sb = [
                const.tile([128, HID], F32, name=f"rs_sb{p}")
                for p in range(N_PHASES)
            ]
            ident = const.tile([128, 128], F32)
            make_identity(nc, ident[:])
            b1k = const.tile([128, 8], F32)
            nc.sync.dma_start(b1k[:], b1.rearrange("(o p) -> p o", p=128))
            b2k = const.tile([128, 8], F32)
            nc.sync.dma_start(b2k[:], b2.rearrange("(o p) -> p o", p=128))
            bhk = const.tile([N_HEAD, 1], F32)
            nc.sync.dma_start(bhk[:], bh[:, None])
            whs = const.tile([128, 8, N_HEAD], F32R)
            nc.sync.dma_start(whs[:], wh3[:])
            outsb = const.tile([N_HEAD, LOCAL_ROWS], F32)

            bounce = [
                dram.tile([PHASE_ROWS, HID], F32, name=f"bounce{p}")
                for p in range(N_PHASES)
            ]
            rs_out = [
                dram.tile([RS_ROWS, HID], F32, name=f"rsout{p}")
                for p in range(N_PHASES)
            ]

            # ---- layer 1: K-split partial matmul, 2 row-phases ----
            # accum is per-phase; phase 1 reuses it after phase 0's spill.
            for ph in range(N_PHASES):
                for ci in range(N_CHUNKS):
                    k0 = ci * CHUNK
                    wts = []
                    for j in range(CHUNK):
                        k = k0 + j
                        if k < K_CACHE:
                            if ph == 0:
                                nc.sync.dma_start(w1c[:, k, :], w13[:, k, :])
                            wts.append(w1c[:, k, :])
                        else:
                            wk = w1s.tile([128, HID], F32R, tag="w1s", name="wk")
                            nc.sync.dma_start(wk[:], w13[:, k, :])
                            wts.append(wk[:])
                    for mi in range(M_PER_PHASE):
                        m = ph * M_PER_PHASE + mi
                        xt = xts.tile([128, CHUNK, 128], F32R, tag="xt")
                        nc.sync.dma_start(
                            xt[:], xT3[:, k0 : k0 + CHUNK, m * 128 : (m + 1) * 128]
                        )
                        pst = [
                            psmm.tile([128, 512], F32, tag=f"mm{n}", name=f"mm{n}")
                            for n in range(2)
                        ]
                        for j in range(CHUNK):
                            for n in range(2):
                                nc.tensor.matmul(
                                    pst[n][:],
                                    xt[:, j, :],
                                    wts[j][:, n * 512 : (n + 1) * 512],
                                    start=(j == 0),
                                    stop=(j == CHUNK - 1),
                                )
                        for n in range(2):
                            dst = accum[ph][:, mi, n * 512 : (n + 1) * 512]
                            if ci == 0:
                                nc.vector.tensor_copy(dst, pst[n][:])
                            else:
                                nc.vector.tensor_add(out=dst, in0=dst, in1=pst[n][:])
                # phase done: spill partial to DRAM, reduce-scatter
                for mi in range(M_PER_PHASE):
                    nc.sync.dma_start(
                        bounce[ph][mi * 128 : (mi + 1) * 128, :], accum[ph][:, mi, :]
                    )
                nc.gpsimd.collective_compute(
                    "ReduceScatter",
                    mybir.AluOpType.add,
                    replica_groups=[list(range(NCORES))],
                    ins=[bounce[ph][:].opt()],
                    outs=[rs_out[ph][:].opt()],
                )
                nc.sync.dma_start(rs_sb[ph][:], rs_out[ph][:])

            # ---- post-RS: transpose -> relu+bias -> hT (after all L1 MMs, so
            # the PE never stalls mid-L1 waiting on a collective) ----
            for ph in range(N_PHASES):
                for o in range(8):
                    tp = psmisc.tile([128, 256], F32, tag="misc", name="tp")[:, :128]
                    nc.tensor.transpose(
                        tp[:], rs_sb[ph][:, o * 128 : (o + 1) * 128], ident[:]
                    )
                    nc.scalar.activation(
                        hT[:, o, ph * RS_ROWS : (ph + 1) * RS_ROWS],
                        tp[:],
                        RELU,
                        bias=b1k[:, o : o + 1],
                    )

            # ---- layer 2: h2T = relu(W2.T @ h1T + b2) ----
            for m2 in range(8):
                wt2 = w2s.tile([128, 8, 128], F32R, tag="w2")
                nc.sync.dma_start(wt2[:], w23[:, :, m2 * 128 : (m2 + 1) * 128])
                ps2 = psmisc.tile([128, LOCAL_ROWS], F32, tag="misc", name="ps2")
                for o in range(8):
                    nc.tensor.matmul(
                        ps2[:],
                        wt2[:, o, :],
                        hT[:, o, :],
                        start=(o == 0),
                        stop=(o == 7),
                    )
                nc.scalar.activation(
                    h2T[:, m2, :], ps2[:], RELU, bias=b2k[:, m2 : m2 + 1]
                )

            # ---- heads: out = Whead.T @ h2T + bhead ----
            psh = psmisc.tile([128, LOCAL_ROWS], F32, tag="misc", name="psh")[:N_HEAD, :]
            for o in range(8):
                nc.tensor.matmul(
                    psh[:],
                    whs[:, o, :],
                    h2T[:, o, :],
                    start=(o == 0),
                    stop=(o == 7),
                )
            nc.scalar.activation(outsb[:], psh[:], IDENT, bias=bhk[:, 0:1])
            nc.sync.dma_start(out[:], outsb[:])

    nc.compile()
    return nc


def _get_nc():
    if "nc" not in _CACHE:
        _CACHE["nc"] = _build()
    return _CACHE["nc"]


def _prep_in_maps(feature_vectors, W1, b1, W2, b2, Wc, bc, Wr, br):
    X = np.asarray(feature_vectors, dtype=np.float32)
    W1 = np.asarray(W1, dtype=np.float32)
    W2 = np.ascontiguousarray(np.asarray(W2, dtype=np.float32))
    b1 = np.ascontiguousarray(np.asarray(b1, dtype=np.float32))
    b2 = np.ascontiguousarray(np.asarray(b2, dtype=np.float32))

    # host-side prep: transpose + row-pad X, concat head weights
    XT = np.zeros((D_IN, ROWS_PAD), dtype=np.float32)
    XT[:, :ROWS] = X.T
    WH = np.ascontiguousarray(
        np.concatenate([np.asarray(Wc), np.asarray(Wr)], axis=1).astype(np.float32)
    )  # [1024, 16]
    BH = np.ascontiguousarray(
        np.concatenate([np.asarray(bc), np.asarray(br)]).astype(np.float32)
    )  # [16]

    return [
        {
            "xT": np.ascontiguousarray(XT[c * KC : (c + 1) * KC]),
            "w1": np.ascontiguousarray(W1[c * KC : (c + 1) * KC]),
            "w2": W2,
            "wh": WH,
            "b1": b1,
            "b2": b2,
            "bh": BH,
        }
        for c in range(NCORES)
    ]


def _stitch(results):
    # stitch: core r, phase p holds padded global rows 1024p + 128r .. +128
    full = np.empty((N_HEAD, ROWS_PAD), dtype=np.float32)
    for r in range(NCORES):
        o = results[r]["out"]
        for p in range(N_PHASES):
            full[:, PHASE_ROWS * p + RS_ROWS * r : PHASE_ROWS * p + RS_ROWS * (r + 1)] = (
                o[:, RS_ROWS * p : RS_ROWS * (p + 1)]
            )
    class_logits = np.ascontiguousarray(full[: C + 1, :ROWS].T)
    box_pred = np.ascontiguousarray(full[C + 1 :, :ROWS].T)
    return class_logits, box_pred


def kernel(feature_vectors, W1, b1, W2, b2, Wc, bc, Wr, br):
    from concourse import bass_utils

    in_maps = _prep_in_maps(feature_vectors, W1, b1, W2, b2, Wc, bc, Wr, br)
    nc = _get_nc()
    res = bass_utils.run_bass_kernel_spmd(nc, in_maps, core_ids=list(range(NCORES)))
    return _stitch(res.results)


# revision 8
# speedup vs baseline: 1.0416x; 1.0416x over previous
"""Trainium2 Bass kernel for nn_BoxHead (nms_detection).

reference:
    h  = relu(X @ W1 + b1)         X: [2000, 50176], W1: [50176, 1024]
    h  = relu(h @ W2 + b2)         W2: [1024, 1024]
    cl = h @ Wc + bc               Wc: [1024, 4]
    bx = h @ Wr + br               Wr: [1024, 12]
    return (cl, bx)

Sharding: contraction-dim (K) split of layer 1 across 8 NeuronCores.
Each core holds X.T rows [6272c:6272(c+1)] (host-transposed, row-padded
to 2048) and the matching W1 row-slice, computes the partial product
partial_c = X_kslice @ W1_kslice -> [2048, 1024] in two row-phases of
1024. After each phase a ReduceScatter(add) leaves each core with 128
fully-reduced rows per phase (256 rows total), on which it runs
bias+relu, layer 2, and both heads (concatenated into one [1024, 16]
weight). The tiny head outputs are returned per-core and stitched on
the host.

Matmuls run as float32r (fp32 storage, single-pass "HIGH" PE mode):
full fp32 DMA bytes, ~4x the fp32 matmul throughput, ~1e-4 rel err.
"""

import numpy as np

NCORES = 8
D_IN = 50176
KC = D_IN // NCORES      # 6272 K-slice per core
KT = KC // 128           # 49 k-tiles of 128
ROWS = 2000
ROWS_PAD = 2048
HID = 1024
N_PHASES = 2
M_TILES = ROWS_PAD // 128            # 16
M_PER_PHASE = M_TILES // N_PHASES    # 8
PHASE_ROWS = 128 * M_PER_PHASE       # 1024
RS_ROWS = PHASE_ROWS // NCORES       # 128 rows per (core, phase)
LOCAL_ROWS = RS_ROWS * N_PHASES      # 256 rows handled per core
K_CACHE = 8                          # k-tiles of W1 kept resident in SBUF
CHUNK = 7                            # k-tiles per PSUM accumulation chunk
N_CHUNKS = KT // CHUNK               # 7
C = 3
N_HEAD = 16                          # 4 class logits + 12 box deltas

_CACHE = {}


def _build():
    import concourse.mybir as mybir
    import concourse.tile as tile
    from concourse import bacc
    from concourse.masks import make_identity

    F32 = mybir.dt.float32
    F32R = mybir.dt.float32r
    RELU = mybir.ActivationFunctionType.Relu
    IDENT = mybir.ActivationFunctionType.Identity

    nc = bacc.Bacc(None, target_bir_lowering=False, debug=False, num_devices=NCORES)

    xT = nc.dram_tensor("xT", [KC, ROWS_PAD], F32R, kind="ExternalInput")
    w1 = nc.dram_tensor("w1", [KC, HID], F32R, kind="ExternalInput")
    w2 = nc.dram_tensor("w2", [HID, HID], F32R, kind="ExternalInput")
    wh = nc.dram_tensor("wh", [HID, N_HEAD], F32R, kind="ExternalInput")
    b1 = nc.dram_tensor("b1", [HID], F32, kind="ExternalInput")
    b2 = nc.dram_tensor("b2", [HID], F32, kind="ExternalInput")
    bh = nc.dram_tensor("bh", [N_HEAD], F32, kind="ExternalInput")
    out = nc.dram_tensor("out", [N_HEAD, LOCAL_ROWS], F32, kind="ExternalOutput")

    xT3 = xT.rearrange("(ko p) m -> p ko m", p=128)    # [128, 49, 2048]
    w13 = w1.rearrange("(ko p) n -> p ko n", p=128)    # [128, 49, 1024]
    w23 = w2.rearrange("(o p) n -> p o n", p=128)      # [128, 8, 1024]
    wh3 = wh.rearrange("(o p) n -> p o n", p=128)      # [128, 8, 16]

    with tile.TileContext(nc) as tc:
        with (
            tc.tile_pool(name="const", bufs=1) as const,
            tc.tile_pool(name="w1s", bufs=10) as w1s,
            tc.tile_pool(name="xts", bufs=5) as xts,
            tc.tile_pool(name="w2s", bufs=2) as w2s,
            tc.tile_pool(name="psmm", bufs=3, space="PSUM") as psmm,
            tc.tile_pool(name="psmisc", bufs=2, space="PSUM") as psmisc,
            tc.tile_pool(name="dram", bufs=1, space="DRAM") as dram,
        ):
            # ---- persistent SBUF ----
            w1c = const.tile([128, K_CACHE, HID], F32R)            # 28KB/part
            accum = [
                const.tile([128, M_PER_PHASE, HID], F32, name=f"accum{p}")
                for p in range(N_PHASES)
            ]
            hT = const.tile([128, 8, LOCAL_ROWS], F32R)            # 8KB/part
            h2T = const.tile([128, 8, LOCAL_ROWS], F32R)           # 8KB/part
            rs_sb = [
                const.tile([128, HID], F32, name=f"rs_sb{p}")
                for p in range(N_PHASES)
            ]
            ident = const.tile([128, 128], F32)
            make_identity(nc, ident[:])
            b1k = const.tile([128, 8], F32)
            nc.sync.dma_start(b1k[:], b1.rearrange("(o p) -> p o", p=128))
            b2k = const.tile([128, 8], F32)
            nc.sync.dma_start(b2k[:], b2.rearrange("(o p) -> p o", p=128))
            bhk = const.tile([N_HEAD, 1], F32)
            nc.sync.dma_start(bhk[:], bh[:, None])
            whs = const.tile([128, 8, N_HEAD], F32R)
            nc.sync.dma_start(whs[:], wh3[:])
            outsb = const.tile([N_HEAD, LOCAL_ROWS], F32)

            bounce = [
                dram.tile([PHASE_ROWS, HID], F32, name=f"bounce{p}")
                for p in range(N_PHASES)
            ]
            rs_out = [
                dram.tile([RS_ROWS, HID], F32, name=f"rsout{p}")
                for p in range(N_PHASES)
            ]

            # ---- layer 1: K-split partial matmul, 2 row-phases ----
            # accum is per-phase; phase 1 reuses it after phase 0's spill.
            for ph in range(N_PHASES):
                for ci in range(N_CHUNKS):
                    k0 = ci * CHUNK
                    wts = []
                    for j in range(CHUNK):
                        k = k0 + j
                        if k < K_CACHE:
                            if ph == 0:
                                nc.sync.dma_start(w1c[:, k, :], w13[:, k, :])
                            wts.append(w1c[:, k, :])
                        else:
                            wk = w1s.tile([128, HID], F32R, tag="w1s", name="wk")
                            nc.sync.dma_start(wk[:], w13[:, k, :])
                            wts.append(wk[:])
                    for mi in range(M_PER_PHASE):
                        m = ph * M_PER_PHASE + mi
                        xt = xts.tile([128, CHUNK, 128], F32R, tag="xt")
                        nc.sync.dma_start(
                            xt[:], xT3[:, k0 : k0 + CHUNK, m * 128 : (m + 1) * 128]
                        )
                        pst = [
                            psmm.tile([128, 512], F32, tag=f"mm{n}", name=f"mm{n}")
                            for n in range(2)
                        ]
                        for j in range(CHUNK):
                            for n in range(2):
                                nc.tensor.matmul(
                                    pst[n][:],
                                    xt[:, j, :],
                                    wts[j][:, n * 512 : (n + 1) * 512],
                                    start=(j == 0),
                                    stop=(j == CHUNK - 1),
                                )
                        for n in range(2):
                            dst = accum[ph][:, mi, n * 512 : (n + 1) * 512]
                            if ci == 0:
                                nc.vector.tensor_copy(dst, pst[n][:])
                            else:
                                nc.vector.tensor_add(out=dst, in0=dst, in1=pst[n][:])
                # phase done: spill partial to DRAM, reduce-scatter
                for mi in range(M_PER_PHASE):
                    nc.sync.dma_start(
                        bounce[ph][mi * 128 : (mi + 1) * 128, :], accum[ph][:, mi, :]
                    )
                nc.gpsimd.collective_compute(
                    "ReduceScatter",
                    mybir.AluOpType.add,
                    replica_groups=[list(range(NCORES))],
                    ins=[bounce[ph][:].opt()],
                    outs=[rs_out[ph][:].opt()],
                )
                nc.gpsimd.dma_start(rs_sb[ph][:], rs_out[ph][:])

            # ---- post-RS: transpose -> relu+bias -> hT (after all L1 MMs, so
            # the PE never stalls mid-L1 waiting on a collective) ----
            for ph in range(N_PHASES):
                for o in range(8):
                    tp = psmisc.tile([128, 256], F32, tag="misc", name="tp")[:, :128]
                    nc.tensor.transpose(
                        tp[:], rs_sb[ph][:, o * 128 : (o + 1) * 128], ident[:]
                    )
                    nc.scalar.activation(
                        hT[:, o, ph * RS_ROWS : (ph + 1) * RS_ROWS],
                        tp[:],
                        RELU,
                        bias=b1k[:, o : o + 1],
                    )

            # ---- layer 2: h2T = relu(W2.T @ h1T + b2) ----
            for m2 in range(8):
                wt2 = w2s.tile([128, 8, 128], F32R, tag="w2")
                nc.sync.dma_start(wt2[:], w23[:, :, m2 * 128 : (m2 + 1) * 128])
                ps2 = psmisc.tile([128, LOCAL_ROWS], F32, tag="misc", name="ps2")
                for o in range(8):
                    nc.tensor.matmul(
                        ps2[:],
                        wt2[:, o, :],
                        hT[:, o, :],
                        start=(o == 0),
                        stop=(o == 7),
                    )
                nc.scalar.activation(
                    h2T[:, m2, :], ps2[:], RELU, bias=b2k[:, m2 : m2 + 1]
                )

            # ---- heads: out = Whead.T @ h2T + bhead ----
            psh = psmisc.tile([128, LOCAL_ROWS], F32, tag="misc", name="psh")[:N_HEAD, :]
            for o in range(8):
                nc.tensor.matmul(
                    psh[:],
                    whs[:, o, :],
                    h2T[:, o, :],
                    start=(o == 0),
                    stop=(o == 7),
                )
            nc.scalar.activation(outsb[:], psh[:], IDENT, bias=bhk[:, 0:1])
            nc.sync.dma_start(out[:], outsb[:])

    nc.compile()
    return nc


def _get_nc():
    if "nc" not in _CACHE:
        _CACHE["nc"] = _build()
    return _CACHE["nc"]


def _prep_in_maps(feature_vectors, W1, b1, W2, b2, Wc, bc, Wr, br):
    X = np.asarray(feature_vectors, dtype=np.float32)
    W1 = np.asarray(W1, dtype=np.float32)
    W2 = np.ascontiguousarray(np.asarray(W2, dtype=np.float32))
    b1 = np.ascontiguousarray(np.asarray(b1, dtype=np.float32))
    b2 = np.ascontiguousarray(np.asarray(b2, dtype=np.float32))

    # host-side prep: transpose + row-pad X, concat head weights
    XT = np.zeros((D_IN, ROWS_PAD), dtype=np.float32)
    XT[:, :ROWS] = X.T
    WH = np.ascontiguousarray(
        np.concatenate([np.asarray(Wc), np.asarray(Wr)], axis=1).astype(np.float32)
    )  # [1024, 16]
    BH = np.ascontiguousarray(
        np.concatenate([np.asarray(bc), np.asarray(br)]).astype(np.float32)
    )  # [16]

    return [
        {
            "xT": np.ascontiguousarray(XT[c * KC : (c + 1) * KC]),
            "w1": np.ascontiguousarray(W1[c * KC : (c + 1) * KC]),
            "w2": W2,
            "wh": WH,
            "b1": b1,
            "b2": b2,
            "bh": BH,
        }
        for c in range(NCORES)
    ]


def _stitch(results):
    # stitch: core r, phase p holds padded global rows 1024p + 128r .. +128
    full = np.empty((N_HEAD, ROWS_PAD), dtype=np.float32)
    for r in range(NCORES):
        o = results[r]["out"]
        for p in range(N_PHASES):
            full[:, PHASE_ROWS * p + RS_ROWS * r : PHASE_ROWS * p + RS_ROWS * (r + 1)] = (
                o[:, RS_ROWS * p : RS_ROWS * (p + 1)]
            )
    class_logits = np.ascontiguousarray(full[: C + 1, :ROWS].T)
    box_pred = np.ascontiguousarray(full[C + 1 :, :ROWS].T)
    return class_logits, box_pred


def kernel(feature_vectors, W1, b1, W2, b2, Wc, bc, Wr, br):
    from concourse import bass_utils

    in_maps = _prep_in_maps(feature_vectors, W1, b1, W2, b2, Wc, bc, Wr, br)
    nc = _get_nc()
    res = bass_utils.run_bass_kernel_spmd(nc, in_maps, core_ids=list(range(NCORES)))
    return _stitch(res.results)
